# revision 17
# baseline (speedup 1.0000x reference)
"""BiAttention (BiDAF-style) Trainium2 kernel.

Full inputs -> shard batch dim over 8 NeuronCores (4 batches each) -> SPMD
Bass/Tile kernel -> gather full output.

Math (per batch), restructured for the hardware (masks are exact {0,1}):
  R'[d,j]   = w_cq[d]*q[j,d] + w_c[d]         (folds w_c, w_cq into rhs)
  sq[j]     = sum_d q[j,d] w_q[d]
  g[j]      = qm[j] * exp(sq[j])              (folds sq + query mask post-exp)
  S0[c,j]   = sum_d c[c,d] R'[d,j]
  en[c,j]   = exp(S0[c,j]) * g[j]             (= qm_j * exp(S[c,j]))
  attn_c2q  = en / sum_j en                   (== reference masked softmax)
  c2q       = (en @ q) / sum_j en             (denominator via ones column)
  mx[c]     = max_j en[c,j]  = exp(masked-max_j S[c,j])
  e2[c]     = cm[c] * mx[c]
  q2c       = (e2 @ c) / sum_c e2             (== reference q2c)
  G         = [c, c2q, c*c2q, c*q2c]
No max-subtraction is needed: |S| <= ~10 for this regime, exp() is safe in f32.

Context tiles are processed in PAIRS (free dim 512) to amortize per-op fixed
costs. Big matmuls run in float32r (TF32-like, 4x faster PE); transposes and
the data path for G's exact columns stay fp32.
"""

import numpy as np

import bass_rust
import concourse.bass as bass
import concourse.mybir as mybir
from concourse.tile import TileContext
from concourse.bass_utils import run_bass_kernel_spmd
from concourse.masks import make_identity

F32 = mybir.dt.float32
F32R = mybir.dt.float32r
AF = mybir.ActivationFunctionType
OP = mybir.AluOpType
AX = mybir.AxisListType

N_CORES = 8
B, C_L, Q_L, D2 = 32, 2048, 256, 256
BPC = B // N_CORES          # batches per core
NP = C_L // 256             # context tile-pairs per batch (pair = 2x128 rows)
NQ = C_L // 512             # context tile-quads per batch (quad = 4x128 rows)
G_W = 4 * D2                # output row width


def _spill_excess_waits(nc, max_waits: int = 1) -> int:
    """The installed walrus rejects >1 sync wait per instruction. Hoist excess
    waits onto same-engine InstNoOp carriers inserted just before."""
    n = 0
    uid = 0
    for f in nc.m.functions:
        for bb in f.blocks:
            out = []
            changed = False
            for inst in bb.instructions:
                si = inst.sync_info
                waits = list(si.on_wait) if si is not None and si.on_wait else []
                if len(waits) > max_waits:
                    head, tail = waits[:-max_waits], waits[-max_waits:]
                    for i in range(0, len(head), max_waits):
                        out.append(
                            mybir.InstNoOp(
                                name=f"I-wspill-{bb.name}-{uid}",
                                engine=inst.engine,
                                ins=[],
                                outs=[],
                                sync_info=bass_rust.SyncInfo(
                                    on_wait=head[i : i + max_waits], on_update=[]
                                ),
                            )
                        )
                        uid += 1
                        n += 1
                    si.on_wait = tail
                    changed = True
                out.append(inst)
            if changed:
                bb.instructions = out
    return n


WORK_BUFS = 4
BATCH_BUFS = 2
PS_PT_BUFS = 2
PS_MM_BUFS = 4


def build_bass():
    nc = bass.Bass()
    ctx_h = nc.declare_dram_parameter("context", [BPC, C_L, D2], F32, isOutput=False)
    cm_h = nc.declare_dram_parameter("context_mask", [BPC, C_L], F32, isOutput=False)
    q_h = nc.declare_dram_parameter("query", [BPC, Q_L, D2], F32, isOutput=False)
    qm_h = nc.declare_dram_parameter("query_mask", [BPC, Q_L], F32, isOutput=False)
    w_h = nc.declare_dram_parameter("W", [3 * D2], F32, isOutput=False)
    g_h = nc.declare_dram_parameter("G", [BPC, C_L, G_W], F32, isOutput=True)

    with TileContext(nc) as tc:
        with (
            tc.tile_pool(name="const", bufs=1) as cpool,
            tc.tile_pool(name="batch", bufs=BATCH_BUFS) as bpool,
            tc.tile_pool(name="cbuf", bufs=2 * NQ + 2) as cpl,
            tc.tile_pool(name="work", bufs=WORK_BUFS) as wpool,
            tc.tile_pool(name="ps_pt", bufs=PS_PT_BUFS, space="PSUM") as ps_pt,
            tc.tile_pool(name="ps_mm", bufs=PS_MM_BUFS, space="PSUM") as ps_mm,
            tc.tile_pool(name="ps_u", bufs=2, space="PSUM") as ps_u,
        ):
            ident = cpool.tile([128, 128], F32)
            make_identity(nc, ident[:])
            ident_r = cpool.tile([128, 128], F32R)
            nc.vector.tensor_copy(ident_r[:], ident[:])
            ones_row = cpool.tile([1, 128], F32)
            nc.vector.memset(ones_row[:], 1.0)
            ones_col = cpool.tile([128, 1], F32)
            nc.vector.memset(ones_col[:], 1.0)
            # W as [128, 6] columns: a=0,1 -> w_c chunks; 2,3 -> w_q; 4,5 -> w_cq
            w6 = cpool.tile([128, 6], F32)
            nc.gpsimd.dma_start(out=w6[:], in_=w_h.rearrange("(a p) -> p a", p=128))

            for b in range(BPC):
                # ---------- per-batch setup ----------
                # q chunks (f32r) with a ones column appended (denominator)
                q_f32 = bpool.tile([128, 2 * D2], F32, tag="q_f32")
                nc.sync.dma_start(
                    out=q_f32[:].rearrange("p (t d) -> p t d", t=2),
                    in_=q_h[b].rearrange("(t p) d -> p t d", p=128),
                )
                q_ext = []
                for jc in range(2):
                    qe = bpool.tile([128, D2 + 4], F32R, tag=f"q_ext{jc}")
                    nc.vector.tensor_copy(qe[:, 0:D2], q_f32[:, jc * D2 : (jc + 1) * D2])
                    nc.vector.memset(qe[:, D2 : D2 + 1].bitcast(F32), 1.0)
                    nc.vector.memset(qe[:, D2 + 1 : D2 + 4].bitcast(F32), 0.0)
                    q_ext.append(qe)

                # qT via 4 PE transposes: qT_sb cols [dc*256, dc*256+256) hold
                # q rows (j) for d-chunk dc
                qT_sb = bpool.tile([128, 2 * Q_L], F32, tag="qT")
                for dc in range(2):
                    qt_ps = ps_pt.tile([128, Q_L], F32, tag="pt")
                    for jc in range(2):
                        nc.tensor.transpose(
                            qt_ps[:, jc * 128 : (jc + 1) * 128],
                            q_f32[:, jc * D2 + dc * 128 : jc * D2 + (dc + 1) * 128],
                            ident[:],
                        )
                    nc.scalar.copy(qT_sb[:, dc * Q_L : (dc + 1) * Q_L], qt_ps[:])

                # R'[dc] = qT*w_cq + w_c (f32r) ; sq = w_q^T @ qT
                Rp = []
                sq_ps = ps_pt.tile([1, Q_L], F32, tag="pt")
                for dc in range(2):
                    rp = bpool.tile([128, Q_L], F32R, tag=f"Rp{dc}")
                    nc.vector.tensor_scalar(
                        out=rp[:],
                        in0=qT_sb[:, dc * Q_L : (dc + 1) * Q_L],
                        scalar1=w6[:, 4 + dc : 5 + dc],
                        scalar2=w6[:, 0 + dc : 1 + dc],
                        op0=OP.mult,
                        op1=OP.add,
                    )
                    Rp.append(rp)
                    nc.tensor.matmul(
                        sq_ps[:],
                        w6[:, 2 + dc : 3 + dc],
                        qT_sb[:, dc * Q_L : (dc + 1) * Q_L],
                        start=(dc == 0),
                        stop=(dc == 1),
                    )

                # g = qm * exp(sq), replicated to 128 partitions via K=1 matmul
                g0 = bpool.tile([1, Q_L], F32, tag="g0")
                nc.scalar.activation(g0[:], sq_ps[:], AF.Exp)
                qm_row = bpool.tile([1, Q_L], F32, tag="qm_row")
                nc.sync.dma_start(
                    out=qm_row[:], in_=qm_h[b].rearrange("(o j) -> o j", o=1)
                )
                g_row = bpool.tile([1, Q_L], F32, tag="g_row")
                nc.vector.tensor_mul(g_row[:], g0[:], qm_row[:])
                g_ps = ps_pt.tile([128, Q_L], F32, tag="pt")
                nc.tensor.matmul(g_ps[:], ones_row[:], g_row[:], start=True, stop=True)
                G128 = bpool.tile([128, Q_L], F32, tag="G128")
                nc.scalar.copy(G128[:], g_ps[:])

                # context mask as [128, 16] (col i = 128-row block i), E2 (f32r)
                CM = bpool.tile([128, 2 * NP], F32, tag="CM")
                nc.sync.dma_start(
                    out=CM[:], in_=cm_h[b].rearrange("(i p) -> p i", p=128)
                )
                E2 = bpool.tile([128, 2 * NP], F32R, tag="E2")

                u2c_ps = ps_u.tile([1, D2], F32, tag="u2c")

                # ---------- main loop over quads of context tiles ----------
                # quad = 4 x 128 context rows; psum-adjacent work runs at pair
                # granularity (PSUM bank limits), SBUF-only ops and DMAs at
                # quad granularity to amortize fixed per-instruction costs.
                c_quads = []
                for p in range(NQ):
                    R0 = p * 512  # first context row of the quad
                    c_quad = cpl.tile([128, 1024], F32, tag="c")
                    nc.sync.dma_start(
                        out=c_quad[:].rearrange("p (t d) -> p t d", t=4),
                        in_=ctx_h[b, R0 : R0 + 512, :].rearrange(
                            "(t p) d -> p t d", p=128
                        ),
                    )
                    c_quads.append(c_quad)
                    # f32r copy of c for the u2c matmul (gpsimd: SBUF->SBUF)
                    c_r = wpool.tile([128, 1024], F32R, tag="c_r")
                    nc.gpsimd.tensor_copy(c_r[:], c_quad[:])

                    en_raw = wpool.tile([128, 1024], F32, tag="en_raw")
                    c2q_quad = wpool.tile([128, 1024], F32, tag="c2q")

                    for h in range(2):  # half = pair of context tiles
                        H0 = h * 512
                        # cT: 4 PE transposes -> one psum bank -> sbuf (f32r)
                        pt_c = ps_pt.tile([128, 512], F32, tag="pt")
                        for o in range(0, 512, 128):
                            nc.tensor.transpose(
                                pt_c[:, o : o + 128].bitcast(F32R),
                                c_r[:, H0 + o : H0 + o + 128],
                                ident_r[:],
                            )
                        cT_sb = wpool.tile([128, 512], F32R, tag="cT")
                        if h == 0:
                            nc.vector.tensor_copy(cT_sb[:], pt_c[:])
                        else:
                            nc.scalar.copy(cT_sb[:], pt_c[:])

                        # S0 for both tiles of the pair into one psum bank
                        s0_ps = ps_mm.tile([128, 512], F32, tag="mm")
                        for t in range(2):
                            for dc in range(2):
                                nc.tensor.matmul(
                                    s0_ps[:, t * 256 : (t + 1) * 256],
                                    cT_sb[
                                        :,
                                        t * 256 + dc * 128 : t * 256 + (dc + 1) * 128,
                                    ],
                                    Rp[dc][:],
                                    start=(dc == 0),
                                    stop=(dc == 1),
                                )
                        nc.scalar.activation(
                            en_raw[:, H0 : H0 + 512], s0_ps[:], AF.Exp
                        )

                    # en = en_raw * g ; mx = per-tile max over j (quad ops)
                    en = wpool.tile([128, 1024], F32R, tag="en")
                    nc.vector.tensor_mul(
                        en[:].rearrange("p (t j) -> p t j", t=4),
                        en_raw[:].rearrange("p (t j) -> p t j", t=4),
                        G128[:].rearrange("p (o j) -> p o j", o=1).broadcast_to(
                            [128, 4, Q_L]
                        ),
                    )
                    mx = wpool.tile([128, 4], F32, tag="mx")
                    nc.vector.tensor_reduce(
                        out=mx[:],
                        in_=en[:].bitcast(F32).rearrange("p (t j) -> p t j", t=4),
                        axis=AX.X,
                        op=OP.max,
                    )
                    nc.vector.tensor_mul(
                        E2[:, 4 * p : 4 * p + 4], mx[:], CM[:, 4 * p : 4 * p + 4]
                    )

                    for h in range(2):
                        H0 = h * 512
                        # enT: 4 PE transposes -> one psum bank -> sbuf
                        pt_e = ps_pt.tile([128, 512], F32, tag="pt")
                        for o in range(0, 512, 128):
                            nc.tensor.transpose(
                                pt_e[:, o : o + 128].bitcast(F32R),
                                en[:, H0 + o : H0 + o + 128],
                                ident_r[:],
                            )
                        enT_sb = wpool.tile([128, 512], F32R, tag="enT")
                        nc.scalar.copy(enT_sb[:], pt_e[:])

                        # c2q per tile (+ denominator in last column); alternate
                        # the normalize-copy between ACT and DVE
                        for t in range(2):
                            c2q_ps = ps_mm.tile([128, D2 + 4], F32, tag="mm")
                            for jc in range(2):
                                nc.tensor.matmul(
                                    c2q_ps[:],
                                    enT_sb[
                                        :,
                                        t * 256 + jc * 128 : t * 256 + (jc + 1) * 128,
                                    ],
                                    q_ext[jc][:],
                                    start=(jc == 0),
                                    stop=(jc == 1),
                                )
                            rcp = wpool.tile([128, 1], F32, tag="rcp")
                            nc.vector.reciprocal(rcp[:], c2q_ps[:, D2 : D2 + 1])
                            dst = c2q_quad[:, H0 + t * 256 : H0 + (t + 1) * 256]
                            if t == 0:
                                nc.scalar.activation(
                                    dst, c2q_ps[:, 0:D2], AF.Identity, scale=rcp[:]
                                )
                            else:
                                nc.vector.tensor_scalar_mul(
                                    dst, c2q_ps[:, 0:D2], rcp[:]
                                )

                        # u2c accumulation (q2c numerator), f32r
                        for t in range(2):
                            tt = 2 * h + t
                            nc.tensor.matmul(
                                u2c_ps[:],
                                E2[:, 4 * p + tt : 4 * p + tt + 1],
                                c_r[:, tt * 256 : (tt + 1) * 256],
                                start=(p == 0 and tt == 0),
                                stop=(p == NQ - 1 and tt == 3),
                            )

                    # cc2q on gpsimd; stream out G columns 0..768 as quad DMAs
                    cc2q = wpool.tile([128, 1024], F32, tag="cc2q")
                    nc.gpsimd.tensor_mul(cc2q[:], c_quad[:], c2q_quad[:])

                    for col0, srct in ((0, c_quad), (D2, c2q_quad), (2 * D2, cc2q)):
                        nc.sync.dma_start(
                            out=g_h[b, R0 : R0 + 512, col0 : col0 + D2].rearrange(
                                "(t p) d -> p t d", p=128
                            ),
                            in_=srct[:].rearrange("p (t d) -> p t d", t=4),
                        )

                # ---------- q2c + phase 2 ----------
                z2 = bpool.tile([128, 1], F32, tag="z2")
                nc.vector.reduce_sum(z2[:], E2[:].bitcast(F32), axis=AX.X)
                z2_ps = ps_pt.tile([1, 1], F32, tag="pt")
                nc.tensor.matmul(z2_ps[:], z2[:], ones_col[:], start=True, stop=True)
                rz = bpool.tile([1, 1], F32, tag="rz")
                nc.vector.reciprocal(rz[:], z2_ps[:])
                q2c_row = bpool.tile([1, D2], F32, tag="q2c_row")
                nc.vector.tensor_scalar_mul(q2c_row[:], u2c_ps[:], rz[:])
                q2c_ps = ps_pt.tile([128, D2], F32, tag="pt")
                nc.tensor.matmul(
                    q2c_ps[:], ones_row[:], q2c_row[:], start=True, stop=True
                )
                Q2C = bpool.tile([128, D2], F32, tag="Q2C")
                nc.scalar.copy(Q2C[:], q2c_ps[:])

                for p in range(NQ):
                    R0 = p * 512
                    cq2c = wpool.tile([128, 1024], F32, tag="cq2c")
                    nc.gpsimd.tensor_mul(
                        cq2c[:].rearrange("p (t d) -> p t d", t=4),
                        c_quads[p][:].rearrange("p (t d) -> p t d", t=4),
                        Q2C[:].rearrange("p (o d) -> p o d", o=1).broadcast_to(
                            [128, 4, D2]
                        ),
                    )
                    nc.sync.dma_start(
                        out=g_h[b, R0 : R0 + 512, 3 * D2 : 4 * D2].rearrange(
                            "(t p) d -> p t d", p=128
                        ),
                        in_=cq2c[:].rearrange("p (t d) -> p t d", t=4),
                    )

    _spill_excess_waits(nc)
    return nc


_NC_CACHE = None


def _get_nc():
    global _NC_CACHE
    if _NC_CACHE is None:
        _NC_CACHE = build_bass()
    return _NC_CACHE


def kernel(**inputs) -> np.ndarray:
    ctx = np.ascontiguousarray(np.asarray(inputs["context"], dtype=np.float32))
    cm = np.ascontiguousarray(np.asarray(inputs["context_mask"], dtype=np.float32))
    q = np.ascontiguousarray(np.asarray(inputs["query"], dtype=np.float32))
    qm = np.ascontiguousarray(np.asarray(inputs["query_mask"], dtype=np.float32))
    w = np.ascontiguousarray(np.asarray(inputs["W"], dtype=np.float32))

    in_maps = []
    for core in range(N_CORES):
        lo, hi = core * BPC, (core + 1) * BPC
        in_maps.append(
            {
                "context": ctx[lo:hi],
                "context_mask": cm[lo:hi],
                "query": q[lo:hi],
                "query_mask": qm[lo:hi],
                "W": w,
            }
        )

    nc = _get_nc()
    res = run_bass_kernel_spmd(nc, in_maps, list(range(N_CORES)))
    return np.concatenate([res.results[i]["G"] for i in range(N_CORES)], axis=0)


# revision 21
# speedup vs baseline: 1.0530x; 1.0530x over previous
"""BiAttention (BiDAF-style) Trainium2 kernel.

Full inputs -> shard batch dim over 8 NeuronCores (4 batches each) -> SPMD
Bass/Tile kernel -> gather full output.

Math (per batch), restructured for the hardware (masks are exact {0,1}):
  R'[d,j]   = w_cq[d]*q[j,d] + w_c[d]         (folds w_c, w_cq into rhs)
  sq[j]     = sum_d q[j,d] w_q[d]
  g[j]      = qm[j] * exp(sq[j])              (folds sq + query mask post-exp)
  S0[c,j]   = sum_d c[c,d] R'[d,j]
  en[c,j]   = exp(S0[c,j]) * g[j]             (= qm_j * exp(S[c,j]))
  attn_c2q  = en / sum_j en                   (== reference masked softmax)
  c2q       = (en @ q) / sum_j en             (denominator via ones column)
  mx[c]     = max_j en[c,j]  = exp(masked-max_j S[c,j])
  e2[c]     = cm[c] * mx[c]
  q2c       = (e2 @ c) / sum_c e2             (== reference q2c)
  G         = [c, c2q, c*c2q, c*q2c]
No max-subtraction is needed: |S| <= ~10 for this regime, exp() is safe in f32.

Context tiles are processed in PAIRS (free dim 512) to amortize per-op fixed
costs. Big matmuls run in float32r (TF32-like, 4x faster PE); transposes and
the data path for G's exact columns stay fp32.
"""

import numpy as np

import bass_rust
import concourse.bass as bass
import concourse.mybir as mybir
from concourse.tile import TileContext
from concourse.bass_utils import run_bass_kernel_spmd
from concourse.masks import make_identity

F32 = mybir.dt.float32
F32R = mybir.dt.float32r
AF = mybir.ActivationFunctionType
OP = mybir.AluOpType
AX = mybir.AxisListType

N_CORES = 8
B, C_L, Q_L, D2 = 32, 2048, 256, 256
BPC = B // N_CORES          # batches per core
NP = C_L // 256             # context tile-pairs per batch (pair = 2x128 rows)
NQ = C_L // 512             # context tile-quads per batch (quad = 4x128 rows)
G_W = 4 * D2                # output row width


def _spill_excess_waits(nc, max_waits: int = 1) -> int:
    """The installed walrus rejects >1 sync wait per instruction. Hoist excess
    waits onto same-engine InstNoOp carriers inserted just before."""
    n = 0
    uid = 0
    for f in nc.m.functions:
        for bb in f.blocks:
            out = []
            changed = False
            for inst in bb.instructions:
                si = inst.sync_info
                waits = list(si.on_wait) if si is not None and si.on_wait else []
                if len(waits) > max_waits:
                    head, tail = waits[:-max_waits], waits[-max_waits:]
                    for i in range(0, len(head), max_waits):
                        out.append(
                            mybir.InstNoOp(
                                name=f"I-wspill-{bb.name}-{uid}",
                                engine=inst.engine,
                                ins=[],
                                outs=[],
                                sync_info=bass_rust.SyncInfo(
                                    on_wait=head[i : i + max_waits], on_update=[]
                                ),
                            )
                        )
                        uid += 1
                        n += 1
                    si.on_wait = tail
                    changed = True
                out.append(inst)
            if changed:
                bb.instructions = out
    return n


WORK_BUFS = 4
BATCH_BUFS = 2
PS_PT_BUFS = 2
PS_MM_BUFS = 4


def build_bass():
    nc = bass.Bass()
    ctx_h = nc.declare_dram_parameter("context", [BPC, C_L, D2], F32, isOutput=False)
    cm_h = nc.declare_dram_parameter("context_mask", [BPC, C_L], F32, isOutput=False)
    q_h = nc.declare_dram_parameter("query", [BPC, Q_L, D2], F32, isOutput=False)
    qm_h = nc.declare_dram_parameter("query_mask", [BPC, Q_L], F32, isOutput=False)
    w_h = nc.declare_dram_parameter("W", [3 * D2], F32, isOutput=False)
    g_h = nc.declare_dram_parameter("G", [BPC, C_L, G_W], F32, isOutput=True)

    with TileContext(nc) as tc:
        with (
            tc.tile_pool(name="const", bufs=1) as cpool,
            tc.tile_pool(name="batch", bufs=BATCH_BUFS) as bpool,
            tc.tile_pool(name="cbuf", bufs=2 * NQ + 2) as cpl,
            tc.tile_pool(name="work", bufs=WORK_BUFS) as wpool,
            tc.tile_pool(name="ps_pt", bufs=PS_PT_BUFS, space="PSUM") as ps_pt,
            tc.tile_pool(name="ps_mm", bufs=PS_MM_BUFS, space="PSUM") as ps_mm,
            tc.tile_pool(name="ps_u", bufs=2, space="PSUM") as ps_u,
        ):
            ident = cpool.tile([128, 128], F32)
            make_identity(nc, ident[:])
            ident_r = cpool.tile([128, 128], F32R)
            nc.vector.tensor_copy(ident_r[:], ident[:])
            ones_row = cpool.tile([1, 128], F32)
            nc.vector.memset(ones_row[:], 1.0)
            ones_col = cpool.tile([128, 1], F32)
            nc.vector.memset(ones_col[:], 1.0)
            # W as [128, 6] columns: a=0,1 -> w_c chunks; 2,3 -> w_q; 4,5 -> w_cq
            w6 = cpool.tile([128, 6], F32)
            nc.gpsimd.dma_start(out=w6[:], in_=w_h.rearrange("(a p) -> p a", p=128))

            for b in range(BPC):
                # ---------- per-batch setup ----------
                # q chunks (f32r) with a ones column appended (denominator)
                q_f32 = bpool.tile([128, 2 * D2], F32, tag="q_f32")
                nc.sync.dma_start(
                    out=q_f32[:].rearrange("p (t d) -> p t d", t=2),
                    in_=q_h[b].rearrange("(t p) d -> p t d", p=128),
                )
                q_ext = []
                for jc in range(2):
                    qe = bpool.tile([128, D2 + 4], F32R, tag=f"q_ext{jc}")
                    nc.vector.tensor_copy(qe[:, 0:D2], q_f32[:, jc * D2 : (jc + 1) * D2])
                    nc.vector.memset(qe[:, D2 : D2 + 1].bitcast(F32), 1.0)
                    nc.vector.memset(qe[:, D2 + 1 : D2 + 4].bitcast(F32), 0.0)
                    q_ext.append(qe)

                # qT via 4 PE transposes: qT_sb cols [dc*256, dc*256+256) hold
                # q rows (j) for d-chunk dc
                qT_sb = bpool.tile([128, 2 * Q_L], F32, tag="qT")
                for dc in range(2):
                    qt_ps = ps_pt.tile([128, Q_L], F32, tag="pt")
                    for jc in range(2):
                        nc.tensor.transpose(
                            qt_ps[:, jc * 128 : (jc + 1) * 128],
                            q_f32[:, jc * D2 + dc * 128 : jc * D2 + (dc + 1) * 128],
                            ident[:],
                        )
                    nc.scalar.copy(qT_sb[:, dc * Q_L : (dc + 1) * Q_L], qt_ps[:])

                # R'[dc] = qT*w_cq + w_c (f32r) ; sq = w_q^T @ qT
                Rp = []
                sq_ps = ps_pt.tile([1, Q_L], F32, tag="pt")
                for dc in range(2):
                    rp = bpool.tile([128, Q_L], F32R, tag=f"Rp{dc}")
                    nc.vector.tensor_scalar(
                        out=rp[:],
                        in0=qT_sb[:, dc * Q_L : (dc + 1) * Q_L],
                        scalar1=w6[:, 4 + dc : 5 + dc],
                        scalar2=w6[:, 0 + dc : 1 + dc],
                        op0=OP.mult,
                        op1=OP.add,
                    )
                    Rp.append(rp)
                    nc.tensor.matmul(
                        sq_ps[:],
                        w6[:, 2 + dc : 3 + dc],
                        qT_sb[:, dc * Q_L : (dc + 1) * Q_L],
                        start=(dc == 0),
                        stop=(dc == 1),
                    )

                # g = qm * exp(sq), replicated to 128 partitions via K=1 matmul
                g0 = bpool.tile([1, Q_L], F32, tag="g0")
                nc.scalar.activation(g0[:], sq_ps[:], AF.Exp)
                qm_row = bpool.tile([1, Q_L], F32, tag="qm_row")
                nc.sync.dma_start(
                    out=qm_row[:], in_=qm_h[b].rearrange("(o j) -> o j", o=1)
                )
                g_row = bpool.tile([1, Q_L], F32, tag="g_row")
                nc.vector.tensor_mul(g_row[:], g0[:], qm_row[:])
                g_ps = ps_pt.tile([128, Q_L], F32, tag="pt")
                nc.tensor.matmul(g_ps[:], ones_row[:], g_row[:], start=True, stop=True)
                G128 = bpool.tile([128, Q_L], F32, tag="G128")
                nc.scalar.copy(G128[:], g_ps[:])

                # context mask as [128, 16] (col i = 128-row block i), E2 (f32r)
                CM = bpool.tile([128, 2 * NP], F32, tag="CM")
                nc.sync.dma_start(
                    out=CM[:], in_=cm_h[b].rearrange("(i p) -> p i", p=128)
                )
                E2 = bpool.tile([128, 2 * NP], F32R, tag="E2")

                u2c_ps = ps_u.tile([1, D2], F32, tag="u2c")

                # ---------- main loop over quads of context tiles ----------
                # quad = 4 x 128 context rows; psum-adjacent work runs at pair
                # granularity (PSUM bank limits), SBUF-only ops and DMAs at
                # quad granularity to amortize fixed per-instruction costs.
                c_quads = []
                for p in range(NQ):
                    R0 = p * 512  # first context row of the quad
                    c_quad = cpl.tile([128, 1024], F32, tag="c")
                    nc.sync.dma_start(
                        out=c_quad[:].rearrange("p (t d) -> p t d", t=4),
                        in_=ctx_h[b, R0 : R0 + 512, :].rearrange(
                            "(t p) d -> p t d", p=128
                        ),
                    )
                    c_quads.append(c_quad)
                    # f32r copy of c for the u2c matmul (gpsimd: SBUF->SBUF)
                    c_r = wpool.tile([128, 1024], F32R, tag="c_r")
                    nc.gpsimd.tensor_copy(c_r[:], c_quad[:])

                    en_raw = wpool.tile([128, 1024], F32, tag="en_raw")
                    c2q_quad = wpool.tile([128, 1024], F32, tag="c2q")

                    for h in range(2):  # half = pair of context tiles
                        H0 = h * 512
                        # cT: 4 PE transposes -> one psum bank -> sbuf (f32r)
                        pt_c = ps_pt.tile([128, 512], F32, tag="pt")
                        for o in range(0, 512, 128):
                            nc.tensor.transpose(
                                pt_c[:, o : o + 128],
                                c_quad[:, H0 + o : H0 + o + 128],
                                ident[:],
                            )
                        cT_sb = wpool.tile([128, 512], F32R, tag="cT")
                        if h == 0:
                            nc.vector.tensor_copy(cT_sb[:], pt_c[:])
                        else:
                            nc.scalar.copy(cT_sb[:], pt_c[:])

                        # S0 for both tiles of the pair into one psum bank
                        s0_ps = ps_mm.tile([128, 512], F32, tag="mm")
                        for t in range(2):
                            for dc in range(2):
                                nc.tensor.matmul(
                                    s0_ps[:, t * 256 : (t + 1) * 256],
                                    cT_sb[
                                        :,
                                        t * 256 + dc * 128 : t * 256 + (dc + 1) * 128,
                                    ],
                                    Rp[dc][:],
                                    start=(dc == 0),
                                    stop=(dc == 1),
                                )
                        nc.scalar.activation(
                            en_raw[:, H0 : H0 + 512], s0_ps[:], AF.Exp
                        )

                    en = wpool.tile([128, 1024], F32R, tag="en")
                    mx = wpool.tile([128, 4], F32, tag="mx")

                    for h in range(2):
                        H0 = h * 512
                        # en = en_raw * g ; mx = per-tile max over j (half ops)
                        nc.vector.tensor_mul(
                            en[:, H0 : H0 + 512].rearrange("p (t j) -> p t j", t=2),
                            en_raw[:, H0 : H0 + 512].rearrange(
                                "p (t j) -> p t j", t=2
                            ),
                            G128[:].rearrange("p (o j) -> p o j", o=1).broadcast_to(
                                [128, 2, Q_L]
                            ),
                        )
                        nc.vector.tensor_reduce(
                            out=mx[:, 2 * h : 2 * h + 2],
                            in_=en[:, H0 : H0 + 512]
                            .bitcast(F32)
                            .rearrange("p (t j) -> p t j", t=2),
                            axis=AX.X,
                            op=OP.max,
                        )
                        nc.vector.tensor_mul(
                            E2[:, 4 * p + 2 * h : 4 * p + 2 * h + 2],
                            mx[:, 2 * h : 2 * h + 2],
                            CM[:, 4 * p + 2 * h : 4 * p + 2 * h + 2],
                        )
                        # enT: 4 PE transposes -> one psum bank -> sbuf
                        pt_e = ps_pt.tile([128, 512], F32, tag="pt")
                        for o in range(0, 512, 128):
                            nc.tensor.transpose(
                                pt_e[:, o : o + 128].bitcast(F32R),
                                en[:, H0 + o : H0 + o + 128],
                                ident_r[:],
                            )
                        enT_sb = wpool.tile([128, 512], F32R, tag="enT")
                        nc.scalar.copy(enT_sb[:], pt_e[:])

                        # c2q per tile (+ denominator in last column); alternate
                        # the normalize-copy between ACT and DVE
                        for t in range(2):
                            c2q_ps = ps_mm.tile([128, D2 + 4], F32, tag="mm")
                            for jc in range(2):
                                nc.tensor.matmul(
                                    c2q_ps[:],
                                    enT_sb[
                                        :,
                                        t * 256 + jc * 128 : t * 256 + (jc + 1) * 128,
                                    ],
                                    q_ext[jc][:],
                                    start=(jc == 0),
                                    stop=(jc == 1),
                                )
                            rcp = wpool.tile([128, 1], F32, tag="rcp")
                            nc.vector.reciprocal(rcp[:], c2q_ps[:, D2 : D2 + 1])
                            dst = c2q_quad[:, H0 + t * 256 : H0 + (t + 1) * 256]
                            if t == 0:
                                nc.scalar.activation(
                                    dst, c2q_ps[:, 0:D2], AF.Identity, scale=rcp[:]
                                )
                            else:
                                nc.vector.tensor_scalar_mul(
                                    dst, c2q_ps[:, 0:D2], rcp[:]
                                )

                        # u2c accumulation (q2c numerator), f32r
                        for t in range(2):
                            tt = 2 * h + t
                            nc.tensor.matmul(
                                u2c_ps[:],
                                E2[:, 4 * p + tt : 4 * p + tt + 1],
                                c_r[:, tt * 256 : (tt + 1) * 256],
                                start=(p == 0 and tt == 0),
                                stop=(p == NQ - 1 and tt == 3),
                            )

                    # cc2q on gpsimd; stream out G columns 0..768 as quad DMAs
                    cc2q = wpool.tile([128, 1024], F32, tag="cc2q")
                    nc.gpsimd.tensor_mul(cc2q[:], c_quad[:], c2q_quad[:])

                    for col0, srct in ((0, c_quad), (D2, c2q_quad), (2 * D2, cc2q)):
                        nc.sync.dma_start(
                            out=g_h[b, R0 : R0 + 512, col0 : col0 + D2].rearrange(
                                "(t p) d -> p t d", p=128
                            ),
                            in_=srct[:].rearrange("p (t d) -> p t d", t=4),
                        )

                # ---------- q2c + phase 2 ----------
                z2 = bpool.tile([128, 1], F32, tag="z2")
                nc.vector.reduce_sum(z2[:], E2[:].bitcast(F32), axis=AX.X)
                z2_ps = ps_pt.tile([1, 1], F32, tag="pt")
                nc.tensor.matmul(z2_ps[:], z2[:], ones_col[:], start=True, stop=True)
                rz = bpool.tile([1, 1], F32, tag="rz")
                nc.vector.reciprocal(rz[:], z2_ps[:])
                q2c_row = bpool.tile([1, D2], F32, tag="q2c_row")
                nc.vector.tensor_scalar_mul(q2c_row[:], u2c_ps[:], rz[:])
                q2c_ps = ps_pt.tile([128, D2], F32, tag="pt")
                nc.tensor.matmul(
                    q2c_ps[:], ones_row[:], q2c_row[:], start=True, stop=True
                )
                Q2C = bpool.tile([128, D2], F32, tag="Q2C")
                nc.scalar.copy(Q2C[:], q2c_ps[:])

                for p in range(NQ):
                    R0 = p * 512
                    cq2c = wpool.tile([128, 1024], F32, tag="cq2c")
                    nc.vector.tensor_mul(
                        cq2c[:].rearrange("p (t d) -> p t d", t=4),
                        c_quads[p][:].rearrange("p (t d) -> p t d", t=4),
                        Q2C[:].rearrange("p (o d) -> p o d", o=1).broadcast_to(
                            [128, 4, D2]
                        ),
                    )
                    nc.sync.dma_start(
                        out=g_h[b, R0 : R0 + 512, 3 * D2 : 4 * D2].rearrange(
                            "(t p) d -> p t d", p=128
                        ),
                        in_=cq2c[:].rearrange("p (t d) -> p t d", t=4),
                    )

    _spill_excess_waits(nc)
    return nc


_NC_CACHE = None


def _get_nc():
    global _NC_CACHE
    if _NC_CACHE is None:
        _NC_CACHE = build_bass()
    return _NC_CACHE


def kernel(**inputs) -> np.ndarray:
    ctx = np.ascontiguousarray(np.asarray(inputs["context"], dtype=np.float32))
    cm = np.ascontiguousarray(np.asarray(inputs["context_mask"], dtype=np.float32))
    q = np.ascontiguousarray(np.asarray(inputs["query"], dtype=np.float32))
    qm = np.ascontiguousarray(np.asarray(inputs["query_mask"], dtype=np.float32))
    w = np.ascontiguousarray(np.asarray(inputs["W"], dtype=np.float32))

    in_maps = []
    for core in range(N_CORES):
        lo, hi = core * BPC, (core + 1) * BPC
        in_maps.append(
            {
                "context": ctx[lo:hi],
                "context_mask": cm[lo:hi],
                "query": q[lo:hi],
                "query_mask": qm[lo:hi],
                "W": w,
            }
        )

    nc = _get_nc()
    res = run_bass_kernel_spmd(nc, in_maps, list(range(N_CORES)))
    return np.concatenate([res.results[i]["G"] for i in range(N_CORES)], axis=0)


# revision 24
# speedup vs baseline: 1.0556x; 1.0024x over previous
"""BiAttention (BiDAF-style) Trainium2 kernel.

Full inputs -> shard batch dim over 8 NeuronCores (4 batches each) -> SPMD
Bass/Tile kernel -> gather full output.

Math (per batch), restructured for the hardware (masks are exact {0,1}):
  R'[d,j]   = w_cq[d]*q[j,d] + w_c[d]         (folds w_c, w_cq into rhs)
  sq[j]     = sum_d q[j,d] w_q[d]
  g[j]      = qm[j] * exp(sq[j])              (folds sq + query mask post-exp)
  S0[c,j]   = sum_d c[c,d] R'[d,j]
  en[c,j]   = exp(S0[c,j]) * g[j]             (= qm_j * exp(S[c,j]))
  attn_c2q  = en / sum_j en                   (== reference masked softmax)
  c2q       = (en @ q) / sum_j en             (denominator via ones column)
  mx[c]     = max_j en[c,j]  = exp(masked-max_j S[c,j])
  e2[c]     = cm[c] * mx[c]
  q2c       = (e2 @ c) / sum_c e2             (== reference q2c)
  G         = [c, c2q, c*c2q, c*q2c]
No max-subtraction is needed: |S| <= ~10 for this regime, exp() is safe in f32.

Context tiles are processed in PAIRS (free dim 512) to amortize per-op fixed
costs. Big matmuls run in float32r (TF32-like, 4x faster PE); transposes and
the data path for G's exact columns stay fp32.
"""

import numpy as np

import bass_rust
import concourse.bass as bass
import concourse.mybir as mybir
from concourse.tile import TileContext
from concourse.bass_utils import run_bass_kernel_spmd
from concourse.masks import make_identity

F32 = mybir.dt.float32
F32R = mybir.dt.float32r
AF = mybir.ActivationFunctionType
OP = mybir.AluOpType
AX = mybir.AxisListType

N_CORES = 8
B, C_L, Q_L, D2 = 32, 2048, 256, 256
BPC = B // N_CORES          # batches per core
NP = C_L // 256             # context tile-pairs per batch (pair = 2x128 rows)
NQ = C_L // 512             # context tile-quads per batch (quad = 4x128 rows)
G_W = 4 * D2                # output row width


def _spill_excess_waits(nc, max_waits: int = 1) -> int:
    """The installed walrus rejects >1 sync wait per instruction. Hoist excess
    waits onto same-engine InstNoOp carriers inserted just before."""
    n = 0
    uid = 0
    for f in nc.m.functions:
        for bb in f.blocks:
            out = []
            changed = False
            for inst in bb.instructions:
                si = inst.sync_info
                waits = list(si.on_wait) if si is not None and si.on_wait else []
                if len(waits) > max_waits:
                    head, tail = waits[:-max_waits], waits[-max_waits:]
                    for i in range(0, len(head), max_waits):
                        out.append(
                            mybir.InstNoOp(
                                name=f"I-wspill-{bb.name}-{uid}",
                                engine=inst.engine,
                                ins=[],
                                outs=[],
                                sync_info=bass_rust.SyncInfo(
                                    on_wait=head[i : i + max_waits], on_update=[]
                                ),
                            )
                        )
                        uid += 1
                        n += 1
                    si.on_wait = tail
                    changed = True
                out.append(inst)
            if changed:
                bb.instructions = out
    return n


WORK_BUFS = 4
BATCH_BUFS = 2
PS_PT_BUFS = 2
PS_MM_BUFS = 4


def build_bass():
    nc = bass.Bass()
    ctx_h = nc.declare_dram_parameter("context", [BPC, C_L, D2], F32, isOutput=False)
    cm_h = nc.declare_dram_parameter("context_mask", [BPC, C_L], F32, isOutput=False)
    q_h = nc.declare_dram_parameter("query", [BPC, Q_L, D2], F32, isOutput=False)
    qm_h = nc.declare_dram_parameter("query_mask", [BPC, Q_L], F32, isOutput=False)
    w_h = nc.declare_dram_parameter("W", [3 * D2], F32, isOutput=False)
    g_h = nc.declare_dram_parameter("G", [BPC, C_L, G_W], F32, isOutput=True)

    with TileContext(nc) as tc:
        with (
            tc.tile_pool(name="const", bufs=1) as cpool,
            tc.tile_pool(name="batch", bufs=BATCH_BUFS) as bpool,
            tc.tile_pool(name="cbuf", bufs=2 * NQ + 2) as cpl,
            tc.tile_pool(name="work", bufs=WORK_BUFS) as wpool,
            tc.tile_pool(name="ps_pt", bufs=PS_PT_BUFS, space="PSUM") as ps_pt,
            tc.tile_pool(name="ps_mm", bufs=PS_MM_BUFS, space="PSUM") as ps_mm,
            tc.tile_pool(name="ps_u", bufs=2, space="PSUM") as ps_u,
        ):
            ident = cpool.tile([128, 128], F32)
            make_identity(nc, ident[:])
            ident_r = cpool.tile([128, 128], F32R)
            nc.vector.tensor_copy(ident_r[:], ident[:])
            ones_row = cpool.tile([1, 128], F32)
            nc.vector.memset(ones_row[:], 1.0)
            ones_col = cpool.tile([128, 1], F32)
            nc.vector.memset(ones_col[:], 1.0)
            # W as [128, 6] columns: a=0,1 -> w_c chunks; 2,3 -> w_q; 4,5 -> w_cq
            w6 = cpool.tile([128, 6], F32)
            nc.gpsimd.dma_start(out=w6[:], in_=w_h.rearrange("(a p) -> p a", p=128))

            for b in range(BPC):
                # ---------- per-batch setup ----------
                # q chunks (f32r) with a ones column appended (denominator)
                q_f32 = bpool.tile([128, 2 * D2], F32, tag="q_f32")
                nc.sync.dma_start(
                    out=q_f32[:].rearrange("p (t d) -> p t d", t=2),
                    in_=q_h[b].rearrange("(t p) d -> p t d", p=128),
                )
                q_ext = []
                for jc in range(2):
                    qe = bpool.tile([128, D2 + 4], F32R, tag=f"q_ext{jc}")
                    nc.vector.tensor_copy(qe[:, 0:D2], q_f32[:, jc * D2 : (jc + 1) * D2])
                    nc.vector.memset(qe[:, D2 : D2 + 1].bitcast(F32), 1.0)
                    nc.vector.memset(qe[:, D2 + 1 : D2 + 4].bitcast(F32), 0.0)
                    q_ext.append(qe)

                # qT via 4 PE transposes: qT_sb cols [dc*256, dc*256+256) hold
                # q rows (j) for d-chunk dc
                qT_sb = bpool.tile([128, 2 * Q_L], F32, tag="qT")
                for dc in range(2):
                    qt_ps = ps_pt.tile([128, Q_L], F32, tag="pt")
                    for jc in range(2):
                        nc.tensor.transpose(
                            qt_ps[:, jc * 128 : (jc + 1) * 128],
                            q_f32[:, jc * D2 + dc * 128 : jc * D2 + (dc + 1) * 128],
                            ident[:],
                        )
                    nc.scalar.copy(qT_sb[:, dc * Q_L : (dc + 1) * Q_L], qt_ps[:])

                # R'[dc] = qT*w_cq + w_c (f32r) ; sq = w_q^T @ qT
                Rp = []
                sq_ps = ps_pt.tile([1, Q_L], F32, tag="pt")
                for dc in range(2):
                    rp = bpool.tile([128, Q_L], F32R, tag=f"Rp{dc}")
                    nc.vector.tensor_scalar(
                        out=rp[:],
                        in0=qT_sb[:, dc * Q_L : (dc + 1) * Q_L],
                        scalar1=w6[:, 4 + dc : 5 + dc],
                        scalar2=w6[:, 0 + dc : 1 + dc],
                        op0=OP.mult,
                        op1=OP.add,
                    )
                    Rp.append(rp)
                    nc.tensor.matmul(
                        sq_ps[:],
                        w6[:, 2 + dc : 3 + dc],
                        qT_sb[:, dc * Q_L : (dc + 1) * Q_L],
                        start=(dc == 0),
                        stop=(dc == 1),
                    )

                # g = qm * exp(sq), replicated to 128 partitions via K=1 matmul
                g0 = bpool.tile([1, Q_L], F32, tag="g0")
                nc.scalar.activation(g0[:], sq_ps[:], AF.Exp)
                qm_row = bpool.tile([1, Q_L], F32, tag="qm_row")
                nc.sync.dma_start(
                    out=qm_row[:], in_=qm_h[b].rearrange("(o j) -> o j", o=1)
                )
                g_row = bpool.tile([1, Q_L], F32, tag="g_row")
                nc.vector.tensor_mul(g_row[:], g0[:], qm_row[:])
                g_ps = ps_pt.tile([128, Q_L], F32, tag="pt")
                nc.tensor.matmul(g_ps[:], ones_row[:], g_row[:], start=True, stop=True)
                G128 = bpool.tile([128, Q_L], F32, tag="G128")
                nc.scalar.copy(G128[:], g_ps[:])

                # context mask as [128, 16] (col i = 128-row block i), E2 (f32r)
                CM = bpool.tile([128, 2 * NP], F32, tag="CM")
                nc.sync.dma_start(
                    out=CM[:], in_=cm_h[b].rearrange("(i p) -> p i", p=128)
                )
                E2 = bpool.tile([128, 2 * NP], F32R, tag="E2")

                u2c_ps = ps_u.tile([1, D2], F32, tag="u2c")

                # ---------- main loop over quads of context tiles ----------
                # quad = 4 x 128 context rows; psum-adjacent work runs at pair
                # granularity (PSUM bank limits), SBUF-only ops and DMAs at
                # quad granularity to amortize fixed per-instruction costs.
                c_quads = []
                for p in range(NQ):
                    R0 = p * 512  # first context row of the quad
                    c_quad = cpl.tile([128, 1024], F32, tag="c")
                    nc.sync.dma_start(
                        out=c_quad[:].rearrange("p (t d) -> p t d", t=4),
                        in_=ctx_h[b, R0 : R0 + 512, :].rearrange(
                            "(t p) d -> p t d", p=128
                        ),
                    )
                    c_quads.append(c_quad)
                    # f32r copy of c for the u2c matmul (gpsimd: SBUF->SBUF)
                    c_r = wpool.tile([128, 1024], F32R, tag="c_r")
                    nc.gpsimd.tensor_copy(c_r[:], c_quad[:])

                    en_raw = wpool.tile([128, 1024], F32, tag="en_raw")
                    c2q_quad = wpool.tile([128, 1024], F32, tag="c2q")

                    for h in range(2):  # half = pair of context tiles
                        H0 = h * 512
                        # cT: 4 PE transposes -> one psum bank -> sbuf (f32r)
                        pt_c = ps_pt.tile([128, 512], F32, tag="pt")
                        for o in range(0, 512, 128):
                            nc.tensor.transpose(
                                pt_c[:, o : o + 128],
                                c_quad[:, H0 + o : H0 + o + 128],
                                ident[:],
                            )
                        cT_sb = wpool.tile([128, 512], F32R, tag="cT")
                        if h == 0:
                            nc.vector.tensor_copy(cT_sb[:], pt_c[:])
                        else:
                            nc.scalar.copy(cT_sb[:], pt_c[:])

                        # S0 for both tiles of the pair into one psum bank
                        s0_ps = ps_mm.tile([128, 512], F32, tag="mm")
                        for t in range(2):
                            for dc in range(2):
                                nc.tensor.matmul(
                                    s0_ps[:, t * 256 : (t + 1) * 256],
                                    cT_sb[
                                        :,
                                        t * 256 + dc * 128 : t * 256 + (dc + 1) * 128,
                                    ],
                                    Rp[dc][:],
                                    start=(dc == 0),
                                    stop=(dc == 1),
                                )
                        nc.scalar.activation(
                            en_raw[:, H0 : H0 + 512], s0_ps[:], AF.Exp
                        )

                    en = wpool.tile([128, 1024], F32R, tag="en")
                    mx = wpool.tile([128, 4], F32, tag="mx")

                    for h in range(2):
                        H0 = h * 512
                        # en = en_raw * g ; mx = per-tile max over j (half ops)
                        nc.vector.tensor_mul(
                            en[:, H0 : H0 + 512].rearrange("p (t j) -> p t j", t=2),
                            en_raw[:, H0 : H0 + 512].rearrange(
                                "p (t j) -> p t j", t=2
                            ),
                            G128[:].rearrange("p (o j) -> p o j", o=1).broadcast_to(
                                [128, 2, Q_L]
                            ),
                        )
                        nc.vector.tensor_reduce(
                            out=mx[:, 2 * h : 2 * h + 2],
                            in_=en[:, H0 : H0 + 512]
                            .bitcast(F32)
                            .rearrange("p (t j) -> p t j", t=2),
                            axis=AX.X,
                            op=OP.max,
                        )
                        nc.vector.tensor_mul(
                            E2[:, 4 * p + 2 * h : 4 * p + 2 * h + 2],
                            mx[:, 2 * h : 2 * h + 2],
                            CM[:, 4 * p + 2 * h : 4 * p + 2 * h + 2],
                        )
                        # enT: 4 PE transposes -> one psum bank -> sbuf
                        pt_e = ps_pt.tile([128, 512], F32, tag="pt")
                        for o in range(0, 512, 128):
                            nc.tensor.transpose(
                                pt_e[:, o : o + 128].bitcast(F32R),
                                en[:, H0 + o : H0 + o + 128],
                                ident_r[:],
                            )
                        enT_sb = wpool.tile([128, 512], F32R, tag="enT")
                        nc.scalar.copy(enT_sb[:], pt_e[:])

                        # c2q per tile (+ denominator in last column); alternate
                        # the normalize-copy between ACT and DVE
                        for t in range(2):
                            c2q_ps = ps_mm.tile([128, D2 + 4], F32, tag="mm")
                            for jc in range(2):
                                nc.tensor.matmul(
                                    c2q_ps[:],
                                    enT_sb[
                                        :,
                                        t * 256 + jc * 128 : t * 256 + (jc + 1) * 128,
                                    ],
                                    q_ext[jc][:],
                                    start=(jc == 0),
                                    stop=(jc == 1),
                                )
                            rcp = wpool.tile([128, 1], F32, tag="rcp")
                            nc.vector.reciprocal(rcp[:], c2q_ps[:, D2 : D2 + 1])
                            dst = c2q_quad[:, H0 + t * 256 : H0 + (t + 1) * 256]
                            if t == 0:
                                nc.scalar.activation(
                                    dst, c2q_ps[:, 0:D2], AF.Identity, scale=rcp[:]
                                )
                            else:
                                nc.vector.tensor_scalar_mul(
                                    dst, c2q_ps[:, 0:D2], rcp[:]
                                )

                        # u2c accumulation (q2c numerator), f32r
                        for t in range(2):
                            tt = 2 * h + t
                            nc.tensor.matmul(
                                u2c_ps[:],
                                E2[:, 4 * p + tt : 4 * p + tt + 1],
                                c_r[:, tt * 256 : (tt + 1) * 256],
                                start=(p == 0 and tt == 0),
                                stop=(p == NQ - 1 and tt == 3),
                            )

                    # cc2q on gpsimd; stream out G columns 0..768 as quad DMAs
                    cc2q = wpool.tile([128, 1024], F32, tag="cc2q")
                    nc.gpsimd.tensor_mul(cc2q[:], c_quad[:], c2q_quad[:])

                    for col0, srct in ((0, c_quad), (D2, c2q_quad), (2 * D2, cc2q)):
                        nc.sync.dma_start(
                            out=g_h[b, R0 : R0 + 512, col0 : col0 + D2].rearrange(
                                "(t p) d -> p t d", p=128
                            ),
                            in_=srct[:].rearrange("p (t d) -> p t d", t=4),
                        )

                # ---------- q2c + phase 2 ----------
                z2 = bpool.tile([128, 1], F32, tag="z2")
                nc.vector.reduce_sum(z2[:], E2[:].bitcast(F32), axis=AX.X)
                z2_ps = ps_pt.tile([1, 1], F32, tag="pt")
                nc.tensor.matmul(z2_ps[:], z2[:], ones_col[:], start=True, stop=True)
                rz = bpool.tile([1, 1], F32, tag="rz")
                nc.vector.reciprocal(rz[:], z2_ps[:])
                q2c_row = bpool.tile([1, D2], F32, tag="q2c_row")
                nc.vector.tensor_scalar_mul(q2c_row[:], u2c_ps[:], rz[:])
                q2c_ps = ps_pt.tile([128, D2], F32, tag="pt")
                nc.tensor.matmul(
                    q2c_ps[:], ones_row[:], q2c_row[:], start=True, stop=True
                )
                Q2C = bpool.tile([128, D2], F32, tag="Q2C")
                nc.scalar.copy(Q2C[:], q2c_ps[:])

                for p in range(NQ):
                    R0 = p * 512
                    cq2c = wpool.tile([128, 1024], F32, tag="cq2c")
                    (nc.vector if p % 2 == 0 else nc.gpsimd).tensor_mul(
                        cq2c[:].rearrange("p (t d) -> p t d", t=4),
                        c_quads[p][:].rearrange("p (t d) -> p t d", t=4),
                        Q2C[:].rearrange("p (o d) -> p o d", o=1).broadcast_to(
                            [128, 4, D2]
                        ),
                    )
                    nc.sync.dma_start(
                        out=g_h[b, R0 : R0 + 512, 3 * D2 : 4 * D2].rearrange(
                            "(t p) d -> p t d", p=128
                        ),
                        in_=cq2c[:].rearrange("p (t d) -> p t d", t=4),
                    )

    _spill_excess_waits(nc)
    return nc


_NC_CACHE = None


def _get_nc():
    global _NC_CACHE
    if _NC_CACHE is None:
        _NC_CACHE = build_bass()
    return _NC_CACHE


def kernel(**inputs) -> np.ndarray:
    ctx = np.ascontiguousarray(np.asarray(inputs["context"], dtype=np.float32))
    cm = np.ascontiguousarray(np.asarray(inputs["context_mask"], dtype=np.float32))
    q = np.ascontiguousarray(np.asarray(inputs["query"], dtype=np.float32))
    qm = np.ascontiguousarray(np.asarray(inputs["query_mask"], dtype=np.float32))
    w = np.ascontiguousarray(np.asarray(inputs["W"], dtype=np.float32))

    in_maps = []
    for core in range(N_CORES):
        lo, hi = core * BPC, (core + 1) * BPC
        in_maps.append(
            {
                "context": ctx[lo:hi],
                "context_mask": cm[lo:hi],
                "query": q[lo:hi],
                "query_mask": qm[lo:hi],
                "W": w,
            }
        )

    nc = _get_nc()
    res = run_bass_kernel_spmd(nc, in_maps, list(range(N_CORES)))
    return np.concatenate([res.results[i]["G"] for i in range(N_CORES)], axis=0)
